# revision 8
# baseline (speedup 1.0000x reference)
"""Trainium2 Bass kernel for nn_AttentionBlock (GroupNorm + 8-head attention
block on [8, 512, 32, 32], residual).

Sharding: pure data-parallel over batch B=8 across the 8 NeuronCores — one
batch element per core, weights replicated, zero collectives.

Per-core dataflow (one batch element, x as [C=512, HW=1024] f32):
  1. GroupNorm(32 groups of 16 channels): raw sums via DVE reduce + ACT square
     accum, cross-partition group combine + expand via tiny matmuls with
     host-provided selector constants, then h = a*x + d on ACT (bf16 out).
  2. proj_in: q, k [512, 1024] (out-channels on partitions) and vT [1024, 520]
     (pixels on partitions; per head 64 v-columns + a constant ones column)
     via bf16 matmuls against host-pre-transposed w_inT.
  3. Per head pair: eT = exp(scale * k_h^T q_h) — the two heads' K=64 logits
     matmuls run concurrently in the two row-halves of the PE array
     (tile_position auto from base partitions); exp on ACT (bf16 out).
     out2[65, 1024] = [vT_h | ones]^T @ eT accumulated over the 8 k-tiles:
     rows 0..63 = unnormalized attention out, row 64 = softmax denominator.
     Reciprocal of the denominator row, then a DRAM round-trip DMA broadcast
     (stride-0 partition dim) to 64 partitions; attn_h = out2*recip + b_v.
  4. proj_out + bias + residual: matmul against host-pre-transposed w_outT,
     then one fused scalar_tensor_tensor: (psum + b_out) + x -> out f32.
"""
import sys

sys.path.insert(0, "/opt/trn_rl_repo")

import numpy as np
import ml_dtypes

import concourse.bass as bass
import concourse.bacc as bacc
import concourse.tile as tile
from concourse import mybir
from concourse.bass_utils import run_bass_kernel_spmd

F32 = mybir.dt.float32
BF16 = mybir.dt.bfloat16
ADD = mybir.AluOpType.add
MULT = mybir.AluOpType.mult

B, C, H, W = 8, 512, 32, 32
HW = H * W       # 1024
NG = 32          # groups
GS = C // NG     # 16 channels per group
NH = 8           # heads
HD = 64          # head dim
HID = NH * HD    # 512
EPS = 1e-6
SCALE = 1.0 / float(np.sqrt(HD))  # 0.125
CT = C // 128    # 4 channel partition-tiles
PT = HW // 128   # 8 pixel partition-tiles
GN_INV = 1.0 / (GS * HW)          # 1/16384


def build_graph():
    nc = bacc.Bacc("TRN2", num_devices=8)

    x_ext = nc.declare_dram_parameter("x", [C, HW], F32, isOutput=False)
    w_inT_ext = nc.declare_dram_parameter("w_inT", [C, 3 * HID], BF16, isOutput=False)
    w_outT_ext = nc.declare_dram_parameter("w_outT", [HID, C], BF16, isOutput=False)
    b_in_ext = nc.declare_dram_parameter("b_in_pm", [128, 12], F32, isOutput=False)
    b_v_ext = nc.declare_dram_parameter("b_v_pm", [HD, NH], F32, isOutput=False)
    b_out_ext = nc.declare_dram_parameter("b_out_pm", [128, CT], F32, isOutput=False)
    gamma_ext = nc.declare_dram_parameter("gamma_pm", [128, CT], F32, isOutput=False)
    beta_ext = nc.declare_dram_parameter("beta_pm", [128, CT], F32, isOutput=False)
    sel_ext = nc.declare_dram_parameter("gn_sel", [128, CT, NG], F32, isOutput=False)
    selT_ext = nc.declare_dram_parameter("gn_selT", [NG, CT, 128], F32, isOutput=False)
    out_ext = nc.declare_dram_parameter("out", [C, HW], F32, isOutput=True)

    recip_dram = nc.dram_tensor("recip_scratch", [NH, HW], F32)

    with tile.TileContext(nc) as tc:
        with (
            tc.tile_pool(name="const", bufs=1) as const,
            tc.tile_pool(name="big", bufs=1) as big,
            tc.tile_pool(name="eT", bufs=1) as eTp,
            tc.tile_pool(name="small", bufs=2) as small,
        ):
            # ---------- loads ----------
            x_sb = [big.tile([128, HW], F32, tag=f"x{t}", name=f"x{t}") for t in range(CT)]
            for t in range(CT):
                nc.gpsimd.dma_start(out=x_sb[t], in_=x_ext[128 * t:128 * (t + 1), :])
            w_inT_sb = [big.tile([128, 3 * HID], BF16, tag=f"wi{t}", name=f"wi{t}") for t in range(CT)]
            for t in range(CT):
                nc.gpsimd.dma_start(out=w_inT_sb[t],
                                    in_=w_inT_ext[128 * t:128 * (t + 1), :])
            w_outT_sb = [big.tile([128, C], BF16, tag=f"wo{t}", name=f"wo{t}") for t in range(CT)]
            for t in range(CT):
                nc.gpsimd.dma_start(out=w_outT_sb[t],
                                    in_=w_outT_ext[128 * t:128 * (t + 1), :])
            b_in_sb = const.tile([128, 12], F32)
            nc.gpsimd.dma_start(out=b_in_sb, in_=b_in_ext[:, :])
            b_v_sb = const.tile([HD, NH], F32)
            nc.gpsimd.dma_start(out=b_v_sb, in_=b_v_ext[:, :])
            b_out_sb = const.tile([128, CT], F32)
            nc.gpsimd.dma_start(out=b_out_sb, in_=b_out_ext[:, :])
            gamma_sb = const.tile([128, CT], F32)
            nc.gpsimd.dma_start(out=gamma_sb, in_=gamma_ext[:, :])
            beta_sb = const.tile([128, CT], F32)
            nc.gpsimd.dma_start(out=beta_sb, in_=beta_ext[:, :])
            sel_sb = const.tile([128, CT, NG], F32)
            nc.gpsimd.dma_start(out=sel_sb, in_=sel_ext[:, :, :])
            selT_sb = const.tile([NG, CT, 128], F32)
            nc.gpsimd.dma_start(out=selT_sb, in_=selT_ext[:, :, :])

            # ---------- groupnorm ----------
            with tc.tile_pool(name="ps_gn", bufs=2, space="PSUM") as ps_gn:
                stats = [small.tile([128, 2], F32, tag=f"st{t}", bufs=1, name=f"st{t}")
                         for t in range(CT)]
                sq_scratch = small.tile([128, HW], F32, tag="sqs", bufs=1)
                for t in range(CT):
                    nc.vector.reduce_sum(stats[t][:, 0:1], x_sb[t][:, :],
                                         axis=mybir.AxisListType.X)
                    nc.scalar.activation(out=sq_scratch, in_=x_sb[t][:, :],
                                         func=mybir.ActivationFunctionType.Square,
                                         accum_out=stats[t][:, 1:2])
                gpsum = ps_gn.tile([NG, 2], F32, tag="gps")
                for t in range(CT):
                    nc.tensor.matmul(gpsum[:, :], lhsT=sel_sb[:, t, :],
                                     rhs=stats[t][:, :],
                                     start=(t == 0), stop=(t == CT - 1))
                # grp cols: 0 rstd, 1 mean*rstd, 2 mean, 3 E[x^2] (later scratch)
                grp = small.tile([NG, 4], F32, tag="grp", bufs=1)
                eps_sb = small.tile([NG, 1], F32, tag="eps_c", bufs=1)
                nc.vector.memset(eps_sb, float(EPS))
                nc.vector.tensor_scalar_mul(grp[:, 2:4], gpsum[:, 0:2], GN_INV)
                nc.vector.tensor_mul(grp[:, 0:1], grp[:, 2:3], grp[:, 2:3])  # mean^2
                nc.vector.tensor_sub(grp[:, 0:1], grp[:, 3:4], grp[:, 0:1])  # var
                nc.scalar.activation(out=grp[:, 0:1], in_=grp[:, 0:1],
                                     func=mybir.ActivationFunctionType.Sqrt,
                                     bias=eps_sb[:, :], scale=1.0)
                nc.vector.reciprocal(out=grp[:, 0:1], in_=grp[:, 0:1])  # rstd
                nc.vector.tensor_mul(grp[:, 1:2], grp[:, 2:3], grp[:, 0:1])
                ga = [small.tile([128, 1], F32, tag=f"ga{t}", bufs=1, name=f"ga{t}")
                      for t in range(CT)]
                gd = [small.tile([128, 1], F32, tag=f"gd{t}", bufs=1, name=f"gd{t}")
                      for t in range(CT)]
                for t in range(CT):
                    epsum = ps_gn.tile([128, 2], F32, tag="eps")
                    nc.tensor.matmul(epsum[:, :], lhsT=selT_sb[:, t, :],
                                     rhs=grp[:, 0:2], start=True, stop=True)
                    nc.vector.tensor_mul(ga[t][:, :], gamma_sb[:, t:t + 1],
                                         epsum[:, 0:1])
                    # d = beta - gamma * (mean*rstd)
                    nc.vector.tensor_mul(gd[t][:, :], gamma_sb[:, t:t + 1],
                                         epsum[:, 1:2])
                    nc.vector.tensor_sub(gd[t][:, :], beta_sb[:, t:t + 1],
                                         gd[t][:, :])
                h_sb = [big.tile([128, HW], BF16, tag=f"h{t}", name=f"h{t}") for t in range(CT)]
                for t in range(CT):
                    nc.scalar.activation(out=h_sb[t], in_=x_sb[t][:, :],
                                         func=mybir.ActivationFunctionType.Identity,
                                         bias=gd[t][:, :], scale=ga[t][:, :])

            # ---------- proj_in ----------
            q_sb = [big.tile([128, HW], BF16, tag=f"q{m}", name=f"q{m}") for m in range(4)]
            k_sb = [big.tile([128, HW], BF16, tag=f"k{m}", name=f"k{m}") for m in range(4)]
            vT_sb = [big.tile([128, NH, HD + 1], BF16, tag=f"vT{p}", name=f"vT{p}")
                     for p in range(PT)]
            with tc.tile_pool(name="ps_pin", bufs=4, space="PSUM") as ps_pin:
                for dest, off in ((q_sb, 0), (k_sb, HID)):
                    for m in range(4):
                        bcol = (off + 128 * m) // 128
                        for n in range(2):
                            pp = ps_pin.tile([128, 512], F32, tag="pp")
                            for t in range(CT):
                                nc.tensor.matmul(
                                    pp[:, :],
                                    lhsT=w_inT_sb[t][:, off + 128 * m:
                                                     off + 128 * (m + 1)],
                                    rhs=h_sb[t][:, 512 * n:512 * (n + 1)],
                                    start=(t == 0), stop=(t == CT - 1))
                            nc.vector.tensor_scalar(
                                out=dest[m][:, 512 * n:512 * (n + 1)], in0=pp[:, :],
                                scalar1=b_in_sb[:, bcol:bcol + 1], scalar2=None,
                                op0=ADD)
                for p in range(PT):
                    nc.vector.memset(vT_sb[p], 1.0)
                for p in range(PT):
                    pp = ps_pin.tile([128, 512], F32, tag="pp")
                    for t in range(CT):
                        nc.tensor.matmul(
                            pp[:, :],
                            lhsT=h_sb[t][:, 128 * p:128 * (p + 1)],
                            rhs=w_inT_sb[t][:, 2 * HID:3 * HID],
                            start=(t == 0), stop=(t == CT - 1))
                    nc.vector.tensor_copy(
                        out=vT_sb[p][:, :, 0:HD],
                        in_=pp[:, :].rearrange("a (nh c) -> a nh c", nh=NH))

            # ---------- attention ----------
            attn_sb = [big.tile([128, HW], BF16, tag=f"at{i}", name=f"at{i}") for i in range(4)]
            with (
                tc.tile_pool(name="ps_log", bufs=2, space="PSUM") as ps_log,
                tc.tile_pool(name="ps_o2", bufs=2, space="PSUM") as ps_o2,
            ):
                for hp in range(4):          # head pairs
                    eTs = []
                    for sub in range(2):
                        eTs.append([eTp.tile([128, HW], BF16, tag=f"eT{sub}_{p}", name=f"eT{sub}_{p}")
                                    for p in range(PT)])
                    # logits + exp, two heads row-tiled concurrently
                    for p in range(PT):
                        pls = []
                        for sub in range(2):
                            lo, hi = 64 * sub, 64 * (sub + 1)
                            pl = ps_log.tile([128, HW], F32, tag="plog")
                            for n in range(2):
                                nc.tensor.matmul(
                                    pl[:, 512 * n:512 * (n + 1)],
                                    lhsT=k_sb[hp][lo:hi, 128 * p:128 * (p + 1)],
                                    rhs=q_sb[hp][lo:hi, 512 * n:512 * (n + 1)],
                                    start=True, stop=True)
                            pls.append(pl)
                        for sub in range(2):
                            nc.scalar.activation(
                                out=eTs[sub][p], in_=pls[sub][:, :],
                                func=mybir.ActivationFunctionType.Exp,
                                scale=SCALE)
                    # out2 + normalize per head
                    for sub in range(2):
                        head = 2 * hp + sub
                        eT = eTs[sub]
                        po = ps_o2.tile([HD + 1, HW], F32, tag="po2")
                        for p in range(PT):
                            for n in range(2):
                                nc.tensor.matmul(
                                    po[:, 512 * n:512 * (n + 1)],
                                    lhsT=vT_sb[p][:, head, :],
                                    rhs=eT[p][:, 512 * n:512 * (n + 1)],
                                    start=(p == 0), stop=(p == PT - 1))
                        # reciprocal of the denom row (PSUM p64 -> SBUF p64)
                        rrow = small.tile([HD + 1, HW], F32, tag="rrow")
                        nc.vector.reciprocal(out=rrow[HD:HD + 1, :],
                                             in_=po[HD:HD + 1, :])
                        nc.gpsimd.dma_start(out=recip_dram[head:head + 1, :],
                                            in_=rrow[HD:HD + 1, :])
                        rb = small.tile([64, HW], F32, tag="rb")
                        bcast_ap = bass.AP(
                            tensor=recip_dram[:, :].tensor,
                            offset=head * HW,
                            ap=[[0, 64], [1, HW]])
                        nc.gpsimd.dma_start(out=rb, in_=bcast_ap)
                        tmp = small.tile([64, HW], BF16, tag="atmp")
                        nc.vector.tensor_mul(tmp[:, :], po[0:HD, :], rb[:, :])
                        if sub == 0:
                            nc.vector.tensor_scalar(
                                out=attn_sb[hp][0:64, :], in0=tmp[:, :],
                                scalar1=b_v_sb[:, head:head + 1],
                                scalar2=None, op0=ADD)
                        else:
                            tmp2 = small.tile([64, HW], BF16, tag="atmp2")
                            nc.vector.tensor_scalar(
                                out=tmp2[:, :], in0=tmp[:, :],
                                scalar1=b_v_sb[:, head:head + 1],
                                scalar2=None, op0=ADD)
                            nc.gpsimd.dma_start(out=attn_sb[hp][64:128, :],
                                                in_=tmp2)

            # ---------- proj_out + bias + residual ----------
            with tc.tile_pool(name="ps_pout", bufs=4, space="PSUM") as ps_pout:
                for m in range(4):
                    for n in range(2):
                        pp = ps_pout.tile([128, 512], F32, tag="pp")
                        for t in range(CT):
                            nc.tensor.matmul(
                                pp[:, :],
                                lhsT=w_outT_sb[t][:, 128 * m:128 * (m + 1)],
                                rhs=attn_sb[t][:, 512 * n:512 * (n + 1)],
                                start=(t == 0), stop=(t == CT - 1))
                        o_sb = small.tile([128, 512], F32, tag="osb")
                        nc.vector.scalar_tensor_tensor(
                            out=o_sb, in0=pp[:, :], scalar=b_out_sb[:, m:m + 1],
                            in1=x_sb[m][:, 512 * n:512 * (n + 1)],
                            op0=ADD, op1=ADD)
                        nc.gpsimd.dma_start(
                            out=out_ext[128 * m:128 * (m + 1),
                                        512 * n:512 * (n + 1)],
                            in_=o_sb)
    return nc


def _install_ntff_hook():
    """The agent image's antenv lacks axon_hooks; synthesize it so
    run_bass_kernel_spmd(trace=True) can reach the NTFF profiler."""
    import types
    if "antenv.axon_hooks" in sys.modules:
        return
    mod = types.ModuleType("antenv.axon_hooks")
    mod._hook = None

    def set_axon_ntff_profile_hook(hook):
        mod._hook = hook

    def get_axon_ntff_profile_hook():
        return mod._hook

    mod.set_axon_ntff_profile_hook = set_axon_ntff_profile_hook
    mod.get_axon_ntff_profile_hook = get_axon_ntff_profile_hook
    sys.modules["antenv.axon_hooks"] = mod
    try:
        from trn_agent_boot.trn_boot import _ntff_profile_via_ctypes
        hook = _ntff_profile_via_ctypes("/opt/axon/libaxon_pjrt.so")
        if hook is not None:
            set_axon_ntff_profile_hook(hook)
    except Exception as e:  # degrade to no tracing
        print("ntff hook setup failed:", e)


_COMPILED = None


def _get_compiled():
    global _COMPILED
    if _COMPILED is None:
        nc = build_graph()
        nc.compile()
        _COMPILED = nc
    return _COMPILED


def _make_consts():
    sel = np.zeros((128, CT, NG), dtype=np.float32)
    selT = np.zeros((NG, CT, 128), dtype=np.float32)
    for t in range(CT):
        for p in range(128):
            g = 8 * t + p // GS
            sel[p, t, g] = 1.0
            selT[g, t, p] = 1.0
    return sel, selT


def _pm(v, cols):
    """[cols*128] vector -> partition-major [128, cols]."""
    return np.ascontiguousarray(v.reshape(cols, 128).T)


def kernel(x, gamma, beta, w_in, b_in, w_out, b_out, _trace=False):
    x = np.asarray(x, dtype=np.float32)
    gamma = np.asarray(gamma, dtype=np.float32)
    beta = np.asarray(beta, dtype=np.float32)
    w_in = np.asarray(w_in, dtype=np.float32)
    b_in = np.asarray(b_in, dtype=np.float32)
    w_out = np.asarray(w_out, dtype=np.float32)
    b_out = np.asarray(b_out, dtype=np.float32)

    w_inT = np.ascontiguousarray(w_in.T).astype(ml_dtypes.bfloat16)
    w_outT = np.ascontiguousarray(w_out.T).astype(ml_dtypes.bfloat16)
    sel, selT = _make_consts()
    b_v = b_in[2 * HID:3 * HID]
    b_v_pm = np.ascontiguousarray(b_v.reshape(NH, HD).T)  # [64, 8]
    common = {
        "w_inT": w_inT,
        "w_outT": w_outT,
        "b_in_pm": _pm(b_in, 12),
        "b_v_pm": b_v_pm,
        "b_out_pm": _pm(b_out, CT),
        "gamma_pm": _pm(gamma, CT),
        "beta_pm": _pm(beta, CT),
        "gn_sel": sel,
        "gn_selT": selT,
    }
    in_maps = []
    for b in range(B):
        m = dict(common)
        m["x"] = np.ascontiguousarray(x[b].reshape(C, HW))
        in_maps.append(m)

    if _trace:
        _install_ntff_hook()
    nc = _get_compiled()
    res = run_bass_kernel_spmd(nc, in_maps, core_ids=list(range(B)),
                               trace=_trace)
    out = np.stack([np.asarray(res.results[b]["out"]).reshape(C, H, W)
                    for b in range(B)])
    if _trace:
        return out, res
    return out


if __name__ == "__main__":
    rng = np.random.default_rng(0)
    inputs = {
        "x": rng.standard_normal((B, C, H, W), dtype=np.float32),
        "gamma": np.ones(C, dtype=np.float32),
        "beta": np.zeros(C, dtype=np.float32),
        "w_in": (rng.standard_normal((3 * HID, C), dtype=np.float32)
                 / np.sqrt(C)),
        "b_in": np.zeros(3 * HID, dtype=np.float32),
        "w_out": (rng.standard_normal((C, HID), dtype=np.float32)
                  / np.sqrt(HID)),
        "b_out": np.zeros(C, dtype=np.float32),
    }
    out = kernel(**inputs)
    print("kernel ran, out shape", out.shape)


# revision 11
# speedup vs baseline: 1.0388x; 1.0388x over previous
"""Trainium2 Bass kernel for nn_AttentionBlock (GroupNorm + 8-head attention
block on [8, 512, 32, 32], residual).

Sharding: pure data-parallel over batch B=8 across the 8 NeuronCores — one
batch element per core, weights replicated, zero collectives.

Per-core dataflow (one batch element, x as [C=512, HW=1024] f32):
  1. GroupNorm(32 groups of 16 channels): raw sums via DVE reduce + ACT square
     accum, cross-partition group combine + expand via tiny matmuls with
     host-provided selector constants, then h = a*x + d on ACT (bf16 out).
  2. proj_in: q, k [512, 1024] (out-channels on partitions) and vT [1024, 520]
     (pixels on partitions; per head 64 v-columns + a constant ones column)
     via bf16 matmuls against host-pre-transposed w_inT.
  3. Per head pair: eT = exp(scale * k_h^T q_h) — the two heads' K=64 logits
     matmuls run concurrently in the two row-halves of the PE array
     (tile_position auto from base partitions); exp on ACT (bf16 out).
     out2[65, 1024] = [vT_h | ones]^T @ eT accumulated over the 8 k-tiles:
     rows 0..63 = unnormalized attention out, row 64 = softmax denominator.
     Reciprocal of the denominator row, then a DRAM round-trip DMA broadcast
     (stride-0 partition dim) to 64 partitions; attn_h = out2*recip + b_v.
  4. proj_out + bias + residual: matmul against host-pre-transposed w_outT,
     then one fused scalar_tensor_tensor: (psum + b_out) + x -> out f32.
"""
import sys

sys.path.insert(0, "/opt/trn_rl_repo")

import numpy as np
import ml_dtypes

import concourse.bass as bass
import concourse.bacc as bacc
import concourse.tile as tile
from concourse import mybir
from concourse.bass_utils import run_bass_kernel_spmd

F32 = mybir.dt.float32
BF16 = mybir.dt.bfloat16
ADD = mybir.AluOpType.add
MULT = mybir.AluOpType.mult

B, C, H, W = 8, 512, 32, 32
HW = H * W       # 1024
NG = 32          # groups
GS = C // NG     # 16 channels per group
NH = 8           # heads
HD = 64          # head dim
HID = NH * HD    # 512
EPS = 1e-6
SCALE = 1.0 / float(np.sqrt(HD))  # 0.125
CT = C // 128    # 4 channel partition-tiles
PT = HW // 128   # 8 pixel partition-tiles
GN_INV = 1.0 / (GS * HW)          # 1/16384


def build_graph():
    nc = bacc.Bacc("TRN2", num_devices=8)

    x_ext = nc.declare_dram_parameter("x", [C, HW], F32, isOutput=False)
    w_inT_ext = nc.declare_dram_parameter("w_inT", [C, 3 * HID], BF16, isOutput=False)
    w_outT_ext = nc.declare_dram_parameter("w_outT", [HID, C], BF16, isOutput=False)
    b_in_ext = nc.declare_dram_parameter("b_in_pm", [128, 12], F32, isOutput=False)
    b_v_ext = nc.declare_dram_parameter("b_v_pm", [HD, NH], F32, isOutput=False)
    b_out_ext = nc.declare_dram_parameter("b_out_pm", [128, CT], F32, isOutput=False)
    gamma_ext = nc.declare_dram_parameter("gamma_pm", [128, CT], F32, isOutput=False)
    beta_ext = nc.declare_dram_parameter("beta_pm", [128, CT], F32, isOutput=False)
    sel_ext = nc.declare_dram_parameter("gn_sel", [128, CT, NG], F32, isOutput=False)
    selT_ext = nc.declare_dram_parameter("gn_selT", [NG, CT, 128], F32, isOutput=False)
    out_ext = nc.declare_dram_parameter("out", [C, HW], F32, isOutput=True)

    recip_dram = nc.dram_tensor("recip_scratch", [NH, HW], F32)

    with tile.TileContext(nc) as tc:
        with (
            tc.tile_pool(name="const", bufs=1) as const,
            tc.tile_pool(name="big", bufs=1) as big,
            tc.tile_pool(name="eT", bufs=1) as eTp,
            tc.tile_pool(name="small", bufs=2) as small,
        ):
            # ---------- loads ----------
            x_sb = [big.tile([128, HW], F32, tag=f"x{t}", name=f"x{t}") for t in range(CT)]
            for t in range(CT):
                nc.gpsimd.dma_start(out=x_sb[t], in_=x_ext[128 * t:128 * (t + 1), :])
            w_inT_sb = [big.tile([128, 3 * HID], BF16, tag=f"wi{t}", name=f"wi{t}") for t in range(CT)]
            for t in range(CT):
                nc.gpsimd.dma_start(out=w_inT_sb[t],
                                    in_=w_inT_ext[128 * t:128 * (t + 1), :])
            w_outT_sb = [big.tile([128, C], BF16, tag=f"wo{t}", name=f"wo{t}") for t in range(CT)]
            for t in range(CT):
                nc.gpsimd.dma_start(out=w_outT_sb[t],
                                    in_=w_outT_ext[128 * t:128 * (t + 1), :])
            b_in_sb = const.tile([128, 12], F32)
            nc.gpsimd.dma_start(out=b_in_sb, in_=b_in_ext[:, :])
            b_v_sb = const.tile([HD, NH], F32)
            nc.gpsimd.dma_start(out=b_v_sb, in_=b_v_ext[:, :])
            b_out_sb = const.tile([128, CT], F32)
            nc.gpsimd.dma_start(out=b_out_sb, in_=b_out_ext[:, :])
            gamma_sb = const.tile([128, CT], F32)
            nc.gpsimd.dma_start(out=gamma_sb, in_=gamma_ext[:, :])
            beta_sb = const.tile([128, CT], F32)
            nc.gpsimd.dma_start(out=beta_sb, in_=beta_ext[:, :])
            sel_sb = const.tile([128, CT, NG], F32)
            nc.gpsimd.dma_start(out=sel_sb, in_=sel_ext[:, :, :])
            selT_sb = const.tile([NG, CT, 128], F32)
            nc.gpsimd.dma_start(out=selT_sb, in_=selT_ext[:, :, :])

            # ---------- groupnorm ----------
            with tc.tile_pool(name="ps_gn", bufs=2, space="PSUM") as ps_gn:
                stats = [small.tile([128, 2], F32, tag=f"st{t}", bufs=1, name=f"st{t}")
                         for t in range(CT)]
                sq_scratch = small.tile([128, HW], F32, tag="sqs", bufs=1)
                for t in range(CT):
                    nc.vector.reduce_sum(stats[t][:, 0:1], x_sb[t][:, :],
                                         axis=mybir.AxisListType.X)
                    nc.scalar.activation(out=sq_scratch, in_=x_sb[t][:, :],
                                         func=mybir.ActivationFunctionType.Square,
                                         accum_out=stats[t][:, 1:2])
                gpsum = ps_gn.tile([NG, 2], F32, tag="gps")
                for t in range(CT):
                    nc.tensor.matmul(gpsum[:, :], lhsT=sel_sb[:, t, :],
                                     rhs=stats[t][:, :],
                                     start=(t == 0), stop=(t == CT - 1))
                # grp cols: 0 rstd, 1 mean*rstd, 2 mean, 3 E[x^2] (later scratch)
                grp = small.tile([NG, 4], F32, tag="grp", bufs=1)
                eps_sb = small.tile([NG, 1], F32, tag="eps_c", bufs=1)
                nc.vector.memset(eps_sb, float(EPS))
                nc.vector.tensor_scalar_mul(grp[:, 2:4], gpsum[:, 0:2], GN_INV)
                nc.vector.tensor_mul(grp[:, 0:1], grp[:, 2:3], grp[:, 2:3])  # mean^2
                nc.vector.tensor_sub(grp[:, 0:1], grp[:, 3:4], grp[:, 0:1])  # var
                nc.scalar.activation(out=grp[:, 0:1], in_=grp[:, 0:1],
                                     func=mybir.ActivationFunctionType.Sqrt,
                                     bias=eps_sb[:, :], scale=1.0)
                nc.vector.reciprocal(out=grp[:, 0:1], in_=grp[:, 0:1])  # rstd
                nc.vector.tensor_mul(grp[:, 1:2], grp[:, 2:3], grp[:, 0:1])
                ga = [small.tile([128, 1], F32, tag=f"ga{t}", bufs=1, name=f"ga{t}")
                      for t in range(CT)]
                gd = [small.tile([128, 1], F32, tag=f"gd{t}", bufs=1, name=f"gd{t}")
                      for t in range(CT)]
                for t in range(CT):
                    epsum = ps_gn.tile([128, 2], F32, tag="eps")
                    nc.tensor.matmul(epsum[:, :], lhsT=selT_sb[:, t, :],
                                     rhs=grp[:, 0:2], start=True, stop=True)
                    nc.vector.tensor_mul(ga[t][:, :], gamma_sb[:, t:t + 1],
                                         epsum[:, 0:1])
                    # d = beta - gamma * (mean*rstd)
                    nc.vector.tensor_mul(gd[t][:, :], gamma_sb[:, t:t + 1],
                                         epsum[:, 1:2])
                    nc.vector.tensor_sub(gd[t][:, :], beta_sb[:, t:t + 1],
                                         gd[t][:, :])
                h_sb = [big.tile([128, HW], BF16, tag=f"h{t}", name=f"h{t}") for t in range(CT)]
                for t in range(CT):
                    nc.scalar.activation(out=h_sb[t], in_=x_sb[t][:, :],
                                         func=mybir.ActivationFunctionType.Identity,
                                         bias=gd[t][:, :], scale=ga[t][:, :])

            # ---------- proj_in ----------
            q_sb = [big.tile([128, HW], BF16, tag=f"q{m}", name=f"q{m}") for m in range(4)]
            k_sb = [big.tile([128, HW], BF16, tag=f"k{m}", name=f"k{m}") for m in range(4)]
            vT_sb = [big.tile([128, NH, HD + 1], BF16, tag=f"vT{p}", name=f"vT{p}")
                     for p in range(PT)]
            with tc.tile_pool(name="ps_pin", bufs=4, space="PSUM") as ps_pin:
                for dest, off in ((q_sb, 0), (k_sb, HID)):
                    for m in range(4):
                        bcol = (off + 128 * m) // 128
                        for n in range(2):
                            pp = ps_pin.tile([128, 512], F32, tag="pp")
                            for t in range(CT):
                                nc.tensor.matmul(
                                    pp[:, :],
                                    lhsT=w_inT_sb[t][:, off + 128 * m:
                                                     off + 128 * (m + 1)],
                                    rhs=h_sb[t][:, 512 * n:512 * (n + 1)],
                                    start=(t == 0), stop=(t == CT - 1))
                            nc.vector.tensor_scalar(
                                out=dest[m][:, 512 * n:512 * (n + 1)], in0=pp[:, :],
                                scalar1=b_in_sb[:, bcol:bcol + 1], scalar2=None,
                                op0=ADD)
                for p in range(PT):
                    nc.vector.memset(vT_sb[p], 1.0)
                for p in range(PT):
                    pp = ps_pin.tile([128, 512], F32, tag="pp")
                    for t in range(CT):
                        nc.tensor.matmul(
                            pp[:, :],
                            lhsT=h_sb[t][:, 128 * p:128 * (p + 1)],
                            rhs=w_inT_sb[t][:, 2 * HID:3 * HID],
                            start=(t == 0), stop=(t == CT - 1))
                    nc.vector.tensor_copy(
                        out=vT_sb[p][:, :, 0:HD],
                        in_=pp[:, :].rearrange("a (nh c) -> a nh c", nh=NH))

            # ---------- attention ----------
            attn_sb = [big.tile([128, HW], BF16, tag=f"at{i}", name=f"at{i}") for i in range(4)]
            with (
                tc.tile_pool(name="ps_log", bufs=2, space="PSUM") as ps_log,
                tc.tile_pool(name="ps_o2", bufs=2, space="PSUM") as ps_o2,
            ):
                eT_all = {}

                def emit_logits_exp(hp):
                    eTs = []
                    for sub in range(2):
                        eTs.append([eTp.tile([128, HW], BF16, bufs=2,
                                             tag=f"eT{sub}_{p}",
                                             name=f"eT{hp}_{sub}_{p}")
                                    for p in range(PT)])
                    eT_all[hp] = eTs
                    for p in range(PT):
                        pls = []
                        for sub in range(2):
                            lo, hi = 64 * sub, 64 * (sub + 1)
                            pl = ps_log.tile([128, HW], F32, tag="plog",
                                             name=f"pl{hp}_{sub}_{p}")
                            for n in range(2):
                                nc.tensor.matmul(
                                    pl[:, 512 * n:512 * (n + 1)],
                                    lhsT=k_sb[hp][lo:hi, 128 * p:128 * (p + 1)],
                                    rhs=q_sb[hp][lo:hi, 512 * n:512 * (n + 1)],
                                    start=True, stop=True)
                            pls.append(pl)
                        for sub in range(2):
                            nc.scalar.activation(
                                out=eTs[sub][p], in_=pls[sub][:, :],
                                func=mybir.ActivationFunctionType.Exp,
                                scale=SCALE)

                def emit_out2_norm(hp):
                    eTs = eT_all.pop(hp)
                    for sub in range(2):
                        head = 2 * hp + sub
                        eT = eTs[sub]
                        po = ps_o2.tile([HD + 1, HW], F32, tag="po2",
                                        name=f"po{head}")
                        for p in range(PT):
                            for n in range(2):
                                nc.tensor.matmul(
                                    po[:, 512 * n:512 * (n + 1)],
                                    lhsT=vT_sb[p][:, head, :],
                                    rhs=eT[p][:, 512 * n:512 * (n + 1)],
                                    start=(p == 0), stop=(p == PT - 1))
                        # early-evict unnormalized out (frees the PSUM slot)
                        attn_u = small.tile([64, HW], BF16, tag="attnu",
                                            bufs=4, name=f"attnu{head}")
                        nc.vector.tensor_copy(out=attn_u, in_=po[0:HD, :])
                        # fast reciprocal of denom row (PSUM p64 -> SBUF p64)
                        rrow = small.tile([HD + 1, HW], F32, tag="rrow",
                                          name=f"rrow{head}")
                        nc.vector.reciprocal(
                            out=rrow[HD:HD + 1, :], in_=po[HD:HD + 1, :])
                        nc.sync.dma_start(out=recip_dram[head:head + 1, :],
                                          in_=rrow[HD:HD + 1, :])
                        rb = small.tile([64, HW], F32, tag="rb",
                                        name=f"rb{head}")
                        bcast_ap = bass.AP(
                            tensor=recip_dram[:, :].tensor,
                            offset=head * HW,
                            ap=[[0, 64], [1, HW]])
                        nc.sync.dma_start(out=rb, in_=bcast_ap)
                        tmp = small.tile([64, HW], BF16, tag="atmp",
                                         name=f"atmp{head}")
                        nc.vector.tensor_mul(tmp[:, :], attn_u[:, :], rb[:, :])
                        if sub == 0:
                            nc.vector.tensor_scalar(
                                out=attn_sb[hp][0:64, :], in0=tmp[:, :],
                                scalar1=b_v_sb[:, head:head + 1],
                                scalar2=None, op0=ADD)
                        else:
                            tmp2 = small.tile([64, HW], BF16, tag="atmp2",
                                              name=f"atmp2{head}")
                            nc.vector.tensor_scalar(
                                out=tmp2[:, :], in0=tmp[:, :],
                                scalar1=b_v_sb[:, head:head + 1],
                                scalar2=None, op0=ADD)
                            nc.sync.dma_start(out=attn_sb[hp][64:128, :],
                                              in_=tmp2)

                # software-pipelined: logits/exp of pair hp overlap
                # out2/normalize of pair hp-1
                for step in range(5):
                    if step < 4:
                        emit_logits_exp(step)
                    if step >= 1:
                        emit_out2_norm(step - 1)

            # ---------- proj_out + bias + residual ----------
            with tc.tile_pool(name="ps_pout", bufs=4, space="PSUM") as ps_pout:
                for m in range(4):
                    for n in range(2):
                        pp = ps_pout.tile([128, 512], F32, tag="pp")
                        for t in range(CT):
                            nc.tensor.matmul(
                                pp[:, :],
                                lhsT=w_outT_sb[t][:, 128 * m:128 * (m + 1)],
                                rhs=attn_sb[t][:, 512 * n:512 * (n + 1)],
                                start=(t == 0), stop=(t == CT - 1))
                        o_sb = small.tile([128, 512], F32, tag="osb")
                        nc.vector.scalar_tensor_tensor(
                            out=o_sb, in0=pp[:, :], scalar=b_out_sb[:, m:m + 1],
                            in1=x_sb[m][:, 512 * n:512 * (n + 1)],
                            op0=ADD, op1=ADD)
                        nc.sync.dma_start(
                            out=out_ext[128 * m:128 * (m + 1),
                                        512 * n:512 * (n + 1)],
                            in_=o_sb)
    return nc


def _install_ntff_hook():
    """The agent image's antenv lacks axon_hooks; synthesize it so
    run_bass_kernel_spmd(trace=True) can reach the NTFF profiler."""
    import types
    if "antenv.axon_hooks" in sys.modules:
        return
    mod = types.ModuleType("antenv.axon_hooks")
    mod._hook = None

    def set_axon_ntff_profile_hook(hook):
        mod._hook = hook

    def get_axon_ntff_profile_hook():
        return mod._hook

    mod.set_axon_ntff_profile_hook = set_axon_ntff_profile_hook
    mod.get_axon_ntff_profile_hook = get_axon_ntff_profile_hook
    sys.modules["antenv.axon_hooks"] = mod
    try:
        from trn_agent_boot.trn_boot import _ntff_profile_via_ctypes
        hook = _ntff_profile_via_ctypes("/opt/axon/libaxon_pjrt.so")
        if hook is not None:
            set_axon_ntff_profile_hook(hook)
    except Exception as e:  # degrade to no tracing
        print("ntff hook setup failed:", e)


_COMPILED = None


def _get_compiled():
    global _COMPILED
    if _COMPILED is None:
        nc = build_graph()
        nc.compile()
        _COMPILED = nc
    return _COMPILED


def _make_consts():
    sel = np.zeros((128, CT, NG), dtype=np.float32)
    selT = np.zeros((NG, CT, 128), dtype=np.float32)
    for t in range(CT):
        for p in range(128):
            g = 8 * t + p // GS
            sel[p, t, g] = 1.0
            selT[g, t, p] = 1.0
    return sel, selT


def _pm(v, cols):
    """[cols*128] vector -> partition-major [128, cols]."""
    return np.ascontiguousarray(v.reshape(cols, 128).T)


def kernel(x, gamma, beta, w_in, b_in, w_out, b_out, _trace=False):
    x = np.asarray(x, dtype=np.float32)
    gamma = np.asarray(gamma, dtype=np.float32)
    beta = np.asarray(beta, dtype=np.float32)
    w_in = np.asarray(w_in, dtype=np.float32)
    b_in = np.asarray(b_in, dtype=np.float32)
    w_out = np.asarray(w_out, dtype=np.float32)
    b_out = np.asarray(b_out, dtype=np.float32)

    w_inT = np.ascontiguousarray(w_in.T).astype(ml_dtypes.bfloat16)
    w_outT = np.ascontiguousarray(w_out.T).astype(ml_dtypes.bfloat16)
    sel, selT = _make_consts()
    b_v = b_in[2 * HID:3 * HID]
    b_v_pm = np.ascontiguousarray(b_v.reshape(NH, HD).T)  # [64, 8]
    common = {
        "w_inT": w_inT,
        "w_outT": w_outT,
        "b_in_pm": _pm(b_in, 12),
        "b_v_pm": b_v_pm,
        "b_out_pm": _pm(b_out, CT),
        "gamma_pm": _pm(gamma, CT),
        "beta_pm": _pm(beta, CT),
        "gn_sel": sel,
        "gn_selT": selT,
    }
    in_maps = []
    for b in range(B):
        m = dict(common)
        m["x"] = np.ascontiguousarray(x[b].reshape(C, HW))
        in_maps.append(m)

    if _trace:
        _install_ntff_hook()
    nc = _get_compiled()
    res = run_bass_kernel_spmd(nc, in_maps, core_ids=list(range(B)),
                               trace=_trace)
    out = np.stack([np.asarray(res.results[b]["out"]).reshape(C, H, W)
                    for b in range(B)])
    if _trace:
        return out, res
    return out


if __name__ == "__main__":
    rng = np.random.default_rng(0)
    inputs = {
        "x": rng.standard_normal((B, C, H, W), dtype=np.float32),
        "gamma": np.ones(C, dtype=np.float32),
        "beta": np.zeros(C, dtype=np.float32),
        "w_in": (rng.standard_normal((3 * HID, C), dtype=np.float32)
                 / np.sqrt(C)),
        "b_in": np.zeros(3 * HID, dtype=np.float32),
        "w_out": (rng.standard_normal((C, HID), dtype=np.float32)
                  / np.sqrt(HID)),
        "b_out": np.zeros(C, dtype=np.float32),
    }
    out = kernel(**inputs)
    print("kernel ran, out shape", out.shape)
